# revision 5
# baseline (speedup 1.0000x reference)
"""Trainium2 Bass kernel for nn_DifferentiableParallelBeamRadon.

Reference op: parallel-beam Radon transform of image [4,1,256,256] over 180
angles -> sinogram [4,1,180,256] (torch-style affine_grid/grid_sample bilinear
sampling with zeros padding, summed over rotated rows, scaled by 2/255).

Strategy
--------
Geometry is input-independent, so at import we precompute, per angle, binned
tap tables: for each (bin P, detector j) a contiguous <=4-cell window base
XIDX[P,j] along the other axis and coefficient planes C[r,P,j] holding the
bilinear weights (reference 2/255 scale folded in).

Exact angle symmetries collapse the weight tables 4-fold: for rep angle
theta in [1,44], the quad {theta, 90-theta, 90+theta, 180-theta} shares one
C table:
  90-theta : same windows at detector 255-j (j-flip)
  90+theta : reflected windows (255-x), reversed taps, transposed image
  180-theta: reflected windows (255-x), reversed taps, same image
(Verified numerically: derived tables match per-angle tables to ~6e-7.)
All flips are absorbed into the host-side gather/output mapping, so the
device applies the IDENTICAL forward C view to all 4 members.

Per slot (= quad group) the cores receive [C | G0 G1 G2 G3] (fp16) as five
DMAs (C first, then one per member, so member-m compute only waits on its
own slice -- pipeline fill is ~1/5 slot): C = [128,(r,h,j)] shared weights,
Gm = [128,(r,h,b,j)] gathered taps per member.  Each core, per member,
computes P = C (*) G with one VectorE tensor_tensor (fp16 2x mode, C
broadcast along the batch dim via a mid-AP step-0 dim) and reduces the 128
partitions (bins) with ones-vector matmuls on TensorE: PSUM limits a single
matmul to 512 f32 out-columns, so each member accumulates 2 chunks of
(b-pair, j) over its (r,h) planes = 4R matmuls; ScalarE drains [1,1024] per
member and DMAs out.

46 groups round-robin (R-sorted) over 8 cores x 6 slots (2 dummy slots).
Per-core traffic 44.6MB (vs 50.5MB unshared) -> DMA-roofline ~134us at
332GB/s/core, with DVE ~85us and PE ~90us hidden underneath.
"""

import os

import numpy as np

IMAGE_SIZE = 256
NUM_ANGLES = 180
NUM_DET = 256
BATCH = 4
N_CORES = 8
R_MAX = 4
PAD = 4
WPAD = IMAGE_SIZE + 2 * PAD  # 264
NMEM = 4

_DT_NP = np.float16

NGROUP = 46
NSLOT = 6  # 48 slot positions, 2 dummies


# ----------------------------------------------------------------------------
# geometry precompute (input independent, cached at import)
# ----------------------------------------------------------------------------

def _angle_tables(a_idx: int):
    """Return (axis, xidx int32 [256,256], C float64 [R_MAX,256,256])."""
    N = IMAGE_SIZE
    angles = np.linspace(0.0, 180.0, NUM_ANGLES + 1, dtype=np.float32)[:-1]
    ang = np.deg2rad(angles[a_idx], dtype=np.float32)
    cos = np.cos(ang, dtype=np.float32)
    sin = np.sin(ang, dtype=np.float32)

    j = np.arange(N, dtype=np.float32)
    xs = ((2.0 * j + 1.0) / np.float32(N) - 1.0).astype(np.float32)
    ys = xs.copy()

    gx = (cos * xs[None, :] + sin * ys[:, None]).astype(np.float32)
    gy = (-sin * xs[None, :] + cos * ys[:, None]).astype(np.float32)
    ix = (((gx + 1.0) * np.float32(N) - 1.0) * np.float32(0.5)).astype(np.float32)
    iy = (((gy + 1.0) * np.float32(N) - 1.0) * np.float32(0.5)).astype(np.float32)

    x0 = np.floor(ix)
    y0 = np.floor(iy)
    wx1 = (ix - x0).astype(np.float64)
    wy1 = (iy - y0).astype(np.float64)
    wx0 = 1.0 - wx1
    wy0 = 1.0 - wy1
    x0 = x0.astype(np.int64)
    y0 = y0.astype(np.int64)

    bin_by_row = abs(float(sin)) <= abs(float(cos))

    taps = [
        (y0, x0, wy0 * wx0),
        (y0, x0 + 1, wy0 * wx1),
        (y0 + 1, x0, wy1 * wx0),
        (y0 + 1, x0 + 1, wy1 * wx1),
    ]

    INF = 1 << 20
    qmin = np.full((N, N), INF, dtype=np.int64)
    qmax = np.full((N, N), -INF, dtype=np.int64)
    jj = np.broadcast_to(np.arange(N)[None, :], (N, N))
    binned = []
    for (rr, cc, w) in taps:
        valid = (rr >= 0) & (rr < N) & (cc >= 0) & (cc < N)
        bp, q = (rr, cc) if bin_by_row else (cc, rr)
        m = valid & (w > 0)
        binned.append((bp, q, w, m))
        np.minimum.at(qmin, (bp[m], jj[m]), q[m])
        np.maximum.at(qmax, (bp[m], jj[m]), q[m])

    width = np.where(qmin <= qmax, qmax - qmin + 1, 0)
    assert width.max() <= R_MAX, f"angle {a_idx}: window {width.max()}"
    qbase = np.where(qmin == INF, 0, qmin)

    C = np.zeros((R_MAX, N, N), dtype=np.float64)
    for (bp, q, w, m) in binned:
        r = q[m] - qbase[bp[m], jj[m]]
        np.add.at(C, (r, bp[m], jj[m]), w[m])

    C *= 2.0 / (IMAGE_SIZE - 1)
    return (0 if bin_by_row else 1), qbase.astype(np.int32), C


_TABLES = None


def _get_tables():
    """Cached group geometry.

    Returns (groups, slot_group, r_slot) where groups[g] =
    (rep, members [(angle, fidx_kind, axis, jflip)], R, xr, Cdev)
    fidx_kind: 0 -> windows [xr, xr+R), 1 -> reflected [255-xr-R+1, ...]
    Cdev: [128, R*2*256] fp16 device-layout shared weights.
    """
    global _TABLES
    if _TABLES is not None:
        return _TABLES

    groups = []
    for rep in range(0, 46):
        axr, xr, Cr = _angle_tables(rep)
        nz = [r for r in range(R_MAX) if np.abs(Cr[r]).max() > 0]
        R = (max(nz) + 1) if nz else 1
        Cr = Cr[:R]
        if rep == 0:
            members = [(0, 0, 0, False), (90, 0, 1, True)]
        elif rep == 45:
            members = [(45, 0, 0, False), (135, 1, 0, False)]
        else:
            members = [
                (rep, 0, 0, False),          # m0: theta
                (90 - rep, 0, 1, True),      # m1: gather fidx0 on axis1, out j-flip
                (90 + rep, 1, 1, False),     # m2: reflected windows, axis1
                (180 - rep, 1, 0, False),    # m3: reflected windows, axis0
            ]
        # device-layout C [pl 128, (r R, h 2, j 256)]
        cl = Cr.reshape(R, 2, 128, NUM_DET).transpose(2, 0, 1, 3)
        cdev = np.ascontiguousarray(cl.reshape(128, -1).astype(_DT_NP))
        groups.append((rep, members, R, xr, cdev))

    r_eff = np.array([g[2] for g in groups])
    order = np.argsort(-r_eff, kind="stable")
    slot_group = np.full((NSLOT, N_CORES), -1, dtype=np.int64)
    for i, g in enumerate(order):
        slot_group[i // N_CORES, i % N_CORES] = g
    r_slot = np.array(
        [max(1, max((r_eff[g] for g in row if g >= 0), default=1))
         for row in slot_group]
    )

    _TABLES = (groups, slot_group, r_slot)
    return _TABLES


# ----------------------------------------------------------------------------
# bass program (built once, cached)
# ----------------------------------------------------------------------------

_PROG = {}


def _build_program(loop: int | None = None):
    """Build (and cache) the Bass program.  loop>1 wraps the body in a
    device-side For_i - timing-measurement only."""
    if loop is None:
        loop = int(os.environ.get("RADON_LOOP", "0"))
    key = loop
    if key in _PROG:
        return _PROG[key]
    import concourse.bacc as bacc
    import concourse.mybir as mybir
    from concourse.tile import TileContext

    _, _, r_slot = _get_tables()

    dt_data = mybir.dt.float16
    LOOP = loop

    c_sizes = [int(r) * 2 * NUM_DET for r in r_slot]          # R*512
    g_sizes = [int(r) * 2 * BATCH * NUM_DET for r in r_slot]  # R*2048 per member
    slot_sizes = [c + NMEM * g for c, g in zip(c_sizes, g_sizes)]
    slot_off = np.concatenate([[0], np.cumsum(slot_sizes)])
    TOT = int(slot_off[-1])
    GMAX = max(g_sizes)
    SLOTMAX = max(slot_sizes)

    nc = bacc.Bacc("TRN2", target_bir_lowering=False, debug=False,
                   num_devices=N_CORES)
    gc_dram = nc.dram_tensor("gc_in", [128, TOT], dt_data,
                             kind="ExternalInput").ap()
    out_dram = nc.dram_tensor("sino_out", [1, NSLOT * NMEM * BATCH * NUM_DET],
                              mybir.dt.float32, kind="ExternalOutput").ap()

    nbj = BATCH * NUM_DET
    with TileContext(nc) as tc:
        with tc.tile_pool(name="const", bufs=1) as cpool, \
             tc.tile_pool(name="gcpool", bufs=2) as gc_pool, \
             tc.tile_pool(name="work", bufs=3) as pool, \
             tc.tile_pool(name="psum", bufs=4, space="PSUM") as psum_pool:
            ones = cpool.tile([128, 1], dt_data)
            nc.vector.memset(ones[:], 1.0)

            def _slot_loop():
                for s in range(NSLOT):
                    Rs = int(r_slot[s])
                    fc = c_sizes[s]
                    fg = g_sizes[s]
                    gc_t = gc_pool.tile([128, SLOTMAX], dt_data, tag="gc")
                    # C first, then one DMA per member: compute on member m
                    # only waits for its own slice (pipeline fill ~1/5 slot)
                    base = slot_off[s]
                    nc.sync.dma_start(
                        out=gc_t[:, :fc], in_=gc_dram[:, base : base + fc]
                    )
                    for m in range(NMEM):
                        o0 = fc + m * fg
                        nc.sync.dma_start(
                            out=gc_t[:, o0 : o0 + fg],
                            in_=gc_dram[:, base + o0 : base + o0 + fg],
                        )
                    c_t = gc_t[:, :fc]
                    c3 = c_t.rearrange("p (r h j) -> p r h j",
                                       r=Rs, h=2, j=NUM_DET)
                    cb = c3.unsqueeze(3).to_broadcast(
                        [128, Rs, 2, BATCH, NUM_DET]
                    )
                    for m in range(NMEM):
                        g_t = gc_t[:, fc + m * fg : fc + (m + 1) * fg]
                        g5 = g_t.rearrange(
                            "p (r h b j) -> p r h b j",
                            r=Rs, h=2, b=BATCH, j=NUM_DET,
                        )
                        p_t = pool.tile([128, GMAX], dt_data, tag="p")
                        p5 = p_t[:, :fg].rearrange(
                            "p (r h b j) -> p r h b j",
                            r=Rs, h=2, b=BATCH, j=NUM_DET,
                        )
                        nc.vector.tensor_mul(out=p5, in0=cb, in1=g5)
                        # reduce 128 bins on PE: psum out <= 512 f32/bank, so
                        # accumulate per b-half over the (r,h) planes
                        ps = psum_pool.tile([1, nbj], mybir.dt.float32,
                                            space="PSUM")
                        nch = nbj // 512  # 2 chunks of (2b,256j)
                        for c in range(nch):
                            for r in range(Rs):
                                for h in range(2):
                                    off = (((r * 2) + h) * nch + c) * 512
                                    nc.tensor.matmul(
                                        out=ps[:, c * 512 : (c + 1) * 512],
                                        lhsT=ones[:],
                                        rhs=p_t[:, off : off + 512],
                                        start=(r == 0 and h == 0),
                                        stop=(r == Rs - 1 and h == 1),
                                    )
                        st = pool.tile([1, nbj], mybir.dt.float32, tag="st")
                        nc.scalar.copy(out=st[:], in_=ps[:])
                        idx = (s * NMEM + m) * nbj
                        nc.scalar.dma_start(
                            out=out_dram[:, idx : idx + nbj], in_=st[:]
                        )

            if LOOP > 1:
                with tc.For_i(0, LOOP, 1):
                    _slot_loop()
            else:
                _slot_loop()

    nc.finalize()
    _PROG[key] = (nc, slot_off, c_sizes, g_sizes, TOT)
    return _PROG[key]


# ----------------------------------------------------------------------------
# entry point
# ----------------------------------------------------------------------------

def _host_pack(img: np.ndarray):
    """img [4,1,256,256] f32 -> per-core packed GC [128, TOT] fp16 arrays."""
    groups, slot_group, r_slot = _get_tables()
    _, slot_off, c_sizes, g_sizes, TOT = _build_program(0)

    im = img[:, 0].astype(np.float32)
    pad0 = np.zeros((BATCH, IMAGE_SIZE, WPAD), dtype=np.float32)
    pad0[:, :, PAD : PAD + IMAGE_SIZE] = im
    pad1 = np.zeros((BATCH, IMAGE_SIZE, WPAD), dtype=np.float32)
    pad1[:, :, PAD : PAD + IMAGE_SIZE] = im.transpose(0, 2, 1)
    flat = [pad0.reshape(BATCH, -1), pad1.reshape(BATCH, -1)]

    pp = np.arange(IMAGE_SIZE)[None, :, None]
    gc_cores = [np.zeros((128, TOT), dtype=_DT_NP) for _ in range(N_CORES)]
    for s in range(NSLOT):
        Rs = int(r_slot[s])
        off = int(slot_off[s])
        fc = c_sizes[s]
        fg = g_sizes[s]
        for k in range(N_CORES):
            g = slot_group[s, k]
            if g < 0:
                continue
            rep, members, R, xr, cdev = groups[g]
            # C (R padded to Rs): [128, (r,h,j)]
            cd = np.zeros((128, Rs, 2, NUM_DET), dtype=_DT_NP)
            cd[:, :R] = cdev.reshape(128, R, 2, NUM_DET)
            gc_cores[k][:, off : off + fc] = cd.reshape(128, -1)
            rr = np.arange(R)[:, None, None]
            fidx = [
                pp * WPAD + (xr[None] + rr + PAD),
                pp * WPAD + (255 - xr[None] - rr + PAD),
            ]
            for m, (ang, kind, axis, jflip) in enumerate(members):
                f = fidx[kind]
                assert f.min() >= 0 and f.max() < IMAGE_SIZE * WPAD
                gm = flat[axis][:, f.ravel()]          # [4, R*256*256]
                gm = gm.reshape(BATCH, R, 2, 128, NUM_DET)
                gd = np.zeros((128, Rs, 2, BATCH, NUM_DET), dtype=np.float32)
                gd[:, :R] = gm.transpose(3, 1, 2, 0, 4)
                gc_cores[k][:, off + fc + m * fg : off + fc + (m + 1) * fg] = (
                    gd.reshape(128, -1).astype(_DT_NP)
                )
    return gc_cores


def kernel(image: np.ndarray, _trace: bool = False):
    from concourse import bass_utils

    image = np.asarray(image)
    nc = _build_program(0)[0]
    groups, slot_group, r_slot = _get_tables()
    gc_cores = _host_pack(image)

    in_maps = [{"gc_in": gc_cores[k]} for k in range(N_CORES)]

    res = bass_utils.run_bass_kernel_spmd(
        nc, in_maps, core_ids=list(range(N_CORES)), trace=_trace
    )

    sino = np.zeros((BATCH, 1, NUM_ANGLES, NUM_DET), dtype=np.float32)
    for k in range(N_CORES):
        o = res.results[k]["sino_out"].reshape(NSLOT, NMEM, BATCH, NUM_DET)
        for s in range(NSLOT):
            g = slot_group[s, k]
            if g < 0:
                continue
            rep, members, R, xr, cdev = groups[g]
            for m, (ang, kind, axis, jflip) in enumerate(members):
                row = o[s, m]
                if jflip:
                    row = row[:, ::-1]
                sino[:, 0, ang, :] = row
    if _trace:
        return sino, res
    return sino


# revision 21
# speedup vs baseline: 1.0275x; 1.0275x over previous
"""Trainium2 Bass kernel for nn_DifferentiableParallelBeamRadon.

Reference op: parallel-beam Radon transform of image [4,1,256,256] over 180
angles -> sinogram [4,1,180,256] (torch-style affine_grid/grid_sample bilinear
sampling with zeros padding, summed over rotated rows, scaled by 2/255).

Strategy
--------
Geometry is input-independent, so at import we precompute, per angle, binned
tap tables: for each (bin P, detector j) a contiguous <=4-cell window base
XIDX[P,j] along the other axis and coefficient planes C[r,P,j] holding the
bilinear weights (reference 2/255 scale folded in).

Exact angle symmetries collapse the weight tables 4-fold: for rep angle
theta in [1,44], the quad {theta, 90-theta, 90+theta, 180-theta} shares one
C table:
  90-theta : same windows at detector 255-j (j-flip)
  90+theta : reflected windows (255-x), reversed taps, transposed image
  180-theta: reflected windows (255-x), reversed taps, same image
(Verified numerically: derived tables match per-angle tables to ~6e-7.)
All flips are absorbed into the host-side gather/output mapping, so the
device applies the IDENTICAL forward C view to all 4 members.

Per slot (= quad group) the cores receive [C | G0 G1 G2 G3] (fp16) as five
DMAs (C first, then one per member, so member-m compute only waits on its
own slice -- pipeline fill is ~1/5 slot): C = [128,(r,h,j)] shared weights,
Gm = [128,(r,h,b,j)] gathered taps per member.  Each core, per member,
computes P = C (*) G with one VectorE tensor_tensor (fp16 2x mode, C
broadcast along the batch dim via a mid-AP step-0 dim) and reduces the 128
partitions (bins) with ones-vector matmuls on TensorE: PSUM limits a single
matmul to 512 f32 out-columns, so each member accumulates 2 chunks of
(b-pair, j) over its (r,h) planes = 4R matmuls; ScalarE drains [1,1024] per
member and DMAs out.

46 groups round-robin (R-sorted) over 8 cores x 6 slots (2 dummy slots).
Per-core traffic 44.6MB (vs 50.5MB unshared) -> DMA-roofline ~134us at
332GB/s/core, with DVE ~85us and PE ~90us hidden underneath.
"""

import os

import numpy as np

IMAGE_SIZE = 256
NUM_ANGLES = 180
NUM_DET = 256
BATCH = 4
N_CORES = 8
R_MAX = 4
PAD = 4
WPAD = IMAGE_SIZE + 2 * PAD  # 264
NMEM = 4

_DT_NP = np.float16

NGROUP = 46
NSLOT = 6  # 48 slot positions, 2 dummies


# ----------------------------------------------------------------------------
# geometry precompute (input independent, cached at import)
# ----------------------------------------------------------------------------

def _angle_tables(a_idx: int):
    """Return (axis, xidx int32 [256,256], C float64 [R_MAX,256,256])."""
    N = IMAGE_SIZE
    angles = np.linspace(0.0, 180.0, NUM_ANGLES + 1, dtype=np.float32)[:-1]
    ang = np.deg2rad(angles[a_idx], dtype=np.float32)
    cos = np.cos(ang, dtype=np.float32)
    sin = np.sin(ang, dtype=np.float32)

    j = np.arange(N, dtype=np.float32)
    xs = ((2.0 * j + 1.0) / np.float32(N) - 1.0).astype(np.float32)
    ys = xs.copy()

    gx = (cos * xs[None, :] + sin * ys[:, None]).astype(np.float32)
    gy = (-sin * xs[None, :] + cos * ys[:, None]).astype(np.float32)
    ix = (((gx + 1.0) * np.float32(N) - 1.0) * np.float32(0.5)).astype(np.float32)
    iy = (((gy + 1.0) * np.float32(N) - 1.0) * np.float32(0.5)).astype(np.float32)

    x0 = np.floor(ix)
    y0 = np.floor(iy)
    wx1 = (ix - x0).astype(np.float64)
    wy1 = (iy - y0).astype(np.float64)
    wx0 = 1.0 - wx1
    wy0 = 1.0 - wy1
    x0 = x0.astype(np.int64)
    y0 = y0.astype(np.int64)

    bin_by_row = abs(float(sin)) <= abs(float(cos))

    taps = [
        (y0, x0, wy0 * wx0),
        (y0, x0 + 1, wy0 * wx1),
        (y0 + 1, x0, wy1 * wx0),
        (y0 + 1, x0 + 1, wy1 * wx1),
    ]

    INF = 1 << 20
    qmin = np.full((N, N), INF, dtype=np.int64)
    qmax = np.full((N, N), -INF, dtype=np.int64)
    jj = np.broadcast_to(np.arange(N)[None, :], (N, N))
    binned = []
    for (rr, cc, w) in taps:
        valid = (rr >= 0) & (rr < N) & (cc >= 0) & (cc < N)
        bp, q = (rr, cc) if bin_by_row else (cc, rr)
        m = valid & (w > 0)
        binned.append((bp, q, w, m))
        np.minimum.at(qmin, (bp[m], jj[m]), q[m])
        np.maximum.at(qmax, (bp[m], jj[m]), q[m])

    width = np.where(qmin <= qmax, qmax - qmin + 1, 0)
    assert width.max() <= R_MAX, f"angle {a_idx}: window {width.max()}"
    qbase = np.where(qmin == INF, 0, qmin)

    C = np.zeros((R_MAX, N, N), dtype=np.float64)
    for (bp, q, w, m) in binned:
        r = q[m] - qbase[bp[m], jj[m]]
        np.add.at(C, (r, bp[m], jj[m]), w[m])

    C *= 2.0 / (IMAGE_SIZE - 1)
    return (0 if bin_by_row else 1), qbase.astype(np.int32), C


_TABLES = None


def _get_tables():
    """Cached group geometry.

    Returns (groups, slot_group, r_slot) where groups[g] =
    (rep, members [(angle, fidx_kind, axis, jflip)], R, xr, Cdev)
    fidx_kind: 0 -> windows [xr, xr+R), 1 -> reflected [255-xr-R+1, ...]
    Cdev: [128, R*2*256] fp16 device-layout shared weights.
    """
    global _TABLES
    if _TABLES is not None:
        return _TABLES

    groups = []
    for rep in range(0, 46):
        axr, xr, Cr = _angle_tables(rep)
        nz = [r for r in range(R_MAX) if np.abs(Cr[r]).max() > 0]
        R = (max(nz) + 1) if nz else 1
        Cr = Cr[:R]
        if rep == 0:
            members = [(0, 0, 0, False), (90, 0, 1, True)]
        elif rep == 45:
            members = [(45, 0, 0, False), (135, 1, 0, False)]
        else:
            members = [
                (rep, 0, 0, False),          # m0: theta
                (90 - rep, 0, 1, True),      # m1: gather fidx0 on axis1, out j-flip
                (90 + rep, 1, 1, False),     # m2: reflected windows, axis1
                (180 - rep, 1, 0, False),    # m3: reflected windows, axis0
            ]
        # device-layout C [pl 128, (r R, h 2, j 256)]
        cl = Cr.reshape(R, 2, 128, NUM_DET).transpose(2, 0, 1, 3)
        cdev = np.ascontiguousarray(cl.reshape(128, -1).astype(_DT_NP))
        groups.append((rep, members, R, xr, cdev))

    r_eff = np.array([g[2] for g in groups])
    order = np.argsort(-r_eff, kind="stable")
    slot_group = np.full((NSLOT, N_CORES), -1, dtype=np.int64)
    for i, g in enumerate(order):
        slot_group[i // N_CORES, i % N_CORES] = g
    r_slot = np.array(
        [max(1, max((r_eff[g] for g in row if g >= 0), default=1))
         for row in slot_group]
    )

    _TABLES = (groups, slot_group, r_slot)
    return _TABLES


# ----------------------------------------------------------------------------
# bass program (built once, cached)
# ----------------------------------------------------------------------------

_PROG = {}


def _build_program(loop: int | None = None):
    """Build (and cache) the Bass program.  loop>1 wraps the body in a
    device-side For_i - timing-measurement only."""
    if loop is None:
        loop = int(os.environ.get("RADON_LOOP", "0"))
    key = loop
    if key in _PROG:
        return _PROG[key]
    import concourse.bacc as bacc
    import concourse.mybir as mybir
    from concourse.tile import TileContext

    _, _, r_slot = _get_tables()

    dt_data = mybir.dt.float16
    LOOP = loop

    c_sizes = [int(r) * 2 * NUM_DET for r in r_slot]          # R*512
    g_sizes = [int(r) * 2 * BATCH * NUM_DET for r in r_slot]  # R*2048 per member
    slot_sizes = [c + NMEM * g for c, g in zip(c_sizes, g_sizes)]
    slot_off = np.concatenate([[0], np.cumsum(slot_sizes)])
    TOT = int(slot_off[-1])
    GMAX = max(g_sizes)
    SLOTMAX = max(slot_sizes)

    nc = bacc.Bacc("TRN2", target_bir_lowering=False, debug=False,
                   num_devices=N_CORES)
    gc_dram = nc.dram_tensor("gc_in", [128, TOT], dt_data,
                             kind="ExternalInput").ap()
    out_dram = nc.dram_tensor("sino_out", [1, NSLOT * NMEM * BATCH * NUM_DET],
                              mybir.dt.float32, kind="ExternalOutput").ap()

    nbj = BATCH * NUM_DET
    with TileContext(nc) as tc:
        with tc.tile_pool(name="const", bufs=1) as cpool, \
             tc.tile_pool(name="gcpool", bufs=2) as gc_pool, \
             tc.tile_pool(name="work", bufs=3) as pool, \
             tc.tile_pool(name="psum", bufs=4, space="PSUM") as psum_pool:
            ones = cpool.tile([128, 1], dt_data)
            nc.vector.memset(ones[:], 1.0)

            def _slot_loop():
                for s in range(NSLOT):
                    Rs = int(r_slot[s])
                    fc = c_sizes[s]
                    fg = g_sizes[s]
                    gc_t = gc_pool.tile([128, SLOTMAX], dt_data, tag="gc")
                    # C first, then one DMA per member: compute on member m
                    # only waits for its own slice (pipeline fill ~1/5 slot).
                    # All input DMAs stay on the SP queue: splitting across
                    # ACT/Pool queues measured worse (head-of-line blocking
                    # behind drains / SWDGE setup).
                    base = slot_off[s]
                    nc.sync.dma_start(
                        out=gc_t[:, :fc], in_=gc_dram[:, base : base + fc]
                    )
                    for m in range(NMEM):
                        o0 = fc + m * fg
                        nc.sync.dma_start(
                            out=gc_t[:, o0 : o0 + fg],
                            in_=gc_dram[:, base + o0 : base + o0 + fg],
                        )
                    c_t = gc_t[:, :fc]
                    c3 = c_t.rearrange("p (r h j) -> p r h j",
                                       r=Rs, h=2, j=NUM_DET)
                    cb = c3.unsqueeze(3).to_broadcast(
                        [128, Rs, 2, BATCH, NUM_DET]
                    )
                    for m in range(NMEM):
                        g_t = gc_t[:, fc + m * fg : fc + (m + 1) * fg]
                        g5 = g_t.rearrange(
                            "p (r h b j) -> p r h b j",
                            r=Rs, h=2, b=BATCH, j=NUM_DET,
                        )
                        p_t = pool.tile([128, GMAX], dt_data, tag="p")
                        p5 = p_t[:, :fg].rearrange(
                            "p (r h b j) -> p r h b j",
                            r=Rs, h=2, b=BATCH, j=NUM_DET,
                        )
                        nc.vector.tensor_mul(out=p5, in0=cb, in1=g5)
                        # reduce 128 bins on PE: psum out <= 512 f32/bank, so
                        # accumulate per b-half over the (r,h) planes
                        ps = psum_pool.tile([1, nbj], mybir.dt.float32,
                                            space="PSUM")
                        nch = nbj // 512  # 2 chunks of (2b,256j)
                        for c in range(nch):
                            for r in range(Rs):
                                for h in range(2):
                                    off = (((r * 2) + h) * nch + c) * 512
                                    nc.tensor.matmul(
                                        out=ps[:, c * 512 : (c + 1) * 512],
                                        lhsT=ones[:],
                                        rhs=p_t[:, off : off + 512],
                                        start=(r == 0 and h == 0),
                                        stop=(r == Rs - 1 and h == 1),
                                    )
                        st = pool.tile([1, nbj], mybir.dt.float32, tag="st")
                        nc.scalar.copy(out=st[:], in_=ps[:])
                        idx = (s * NMEM + m) * nbj
                        nc.scalar.dma_start(
                            out=out_dram[:, idx : idx + nbj], in_=st[:]
                        )

            if LOOP > 1:
                with tc.For_i(0, LOOP, 1):
                    _slot_loop()
            else:
                _slot_loop()

    nc.finalize()
    _PROG[key] = (nc, slot_off, c_sizes, g_sizes, TOT)
    return _PROG[key]


# ----------------------------------------------------------------------------
# entry point
# ----------------------------------------------------------------------------

def _host_pack(img: np.ndarray):
    """img [4,1,256,256] f32 -> per-core packed GC [128, TOT] fp16 arrays."""
    groups, slot_group, r_slot = _get_tables()
    _, slot_off, c_sizes, g_sizes, TOT = _build_program(0)

    im = img[:, 0].astype(np.float32)
    pad0 = np.zeros((BATCH, IMAGE_SIZE, WPAD), dtype=np.float32)
    pad0[:, :, PAD : PAD + IMAGE_SIZE] = im
    pad1 = np.zeros((BATCH, IMAGE_SIZE, WPAD), dtype=np.float32)
    pad1[:, :, PAD : PAD + IMAGE_SIZE] = im.transpose(0, 2, 1)
    flat = [pad0.reshape(BATCH, -1), pad1.reshape(BATCH, -1)]

    pp = np.arange(IMAGE_SIZE)[None, :, None]
    gc_cores = [np.zeros((128, TOT), dtype=_DT_NP) for _ in range(N_CORES)]
    for s in range(NSLOT):
        Rs = int(r_slot[s])
        off = int(slot_off[s])
        fc = c_sizes[s]
        fg = g_sizes[s]
        for k in range(N_CORES):
            g = slot_group[s, k]
            if g < 0:
                continue
            rep, members, R, xr, cdev = groups[g]
            # C (R padded to Rs): [128, (r,h,j)]
            cd = np.zeros((128, Rs, 2, NUM_DET), dtype=_DT_NP)
            cd[:, :R] = cdev.reshape(128, R, 2, NUM_DET)
            gc_cores[k][:, off : off + fc] = cd.reshape(128, -1)
            rr = np.arange(R)[:, None, None]
            fidx = [
                pp * WPAD + (xr[None] + rr + PAD),
                pp * WPAD + (255 - xr[None] - rr + PAD),
            ]
            for m, (ang, kind, axis, jflip) in enumerate(members):
                f = fidx[kind]
                assert f.min() >= 0 and f.max() < IMAGE_SIZE * WPAD
                gm = flat[axis][:, f.ravel()]          # [4, R*256*256]
                gm = gm.reshape(BATCH, R, 2, 128, NUM_DET)
                gd = np.zeros((128, Rs, 2, BATCH, NUM_DET), dtype=np.float32)
                gd[:, :R] = gm.transpose(3, 1, 2, 0, 4)
                gc_cores[k][:, off + fc + m * fg : off + fc + (m + 1) * fg] = (
                    gd.reshape(128, -1).astype(_DT_NP)
                )
    return gc_cores


def kernel(image: np.ndarray, _trace: bool = False):
    from concourse import bass_utils

    image = np.asarray(image)
    nc = _build_program(0)[0]
    groups, slot_group, r_slot = _get_tables()
    gc_cores = _host_pack(image)

    in_maps = [{"gc_in": gc_cores[k]} for k in range(N_CORES)]

    res = bass_utils.run_bass_kernel_spmd(
        nc, in_maps, core_ids=list(range(N_CORES)), trace=_trace
    )

    sino = np.zeros((BATCH, 1, NUM_ANGLES, NUM_DET), dtype=np.float32)
    for k in range(N_CORES):
        o = res.results[k]["sino_out"].reshape(NSLOT, NMEM, BATCH, NUM_DET)
        for s in range(NSLOT):
            g = slot_group[s, k]
            if g < 0:
                continue
            rep, members, R, xr, cdev = groups[g]
            for m, (ang, kind, axis, jflip) in enumerate(members):
                row = o[s, m]
                if jflip:
                    row = row[:, ::-1]
                sino[:, 0, ang, :] = row
    if _trace:
        return sino, res
    return sino
